# revision 1
# baseline (speedup 1.0000x reference)
"""Trainium2 Bass kernel for BERTForContrastiveLearningForTokenMetric loss.

Math: the reference loss factors into masked per-token sums:
    proto = (sum_{ent} x_t) / n_ent
    loss  = (sum_{nz} x_t/||x_t||) . proto / (||proto|| * n_tok)
so one pass over logits per core suffices.  Each core processes 8 of the 64
batches (4096 tokens), producing a [2, 768] partial:
    row 0 = sum_{ent tokens} x_t
    row 1 = sum_{nz tokens}  x_t / ||x_t||
The host sums partials across the 8 cores and does the tiny final combine.

Per-core device pipeline, per 512-token block (tokens laid out 4/partition):
    DMA 1.5 MiB x-block -> SBUF [128, 4, 768]
    DVE tensor_tensor_reduce (x*x, sum) -> sq [128, 4]     (per-token norms^2)
    DVE reciprocal -> 1/sq; ACT sqrt -> 1/||x||
    DVE mult (in place): aux nz slot <- nz / ||x||         (matmul weights)
    PE  matmul lhsT=[128,2] (ent, nz/||x||), rhs=x -> PSUM [2,768] accumulate
"""

import numpy as np

B, S, D = 64, 512, 768
N_CORES = 8
B_PER_CORE = B // N_CORES            # 8
TOK_PER_CORE = B_PER_CORE * S        # 4096
P = 128                              # SBUF partitions
J = 4                                # tokens per partition per block
BLK_TOK = P * J                      # 512 tokens per block
N_BLK = TOK_PER_CORE // BLK_TOK      # 8

_CACHE = {}


def _tile_program(nc, x_h, aux_h, out_h, repeat=1):
    """Emit the per-core Tile program.

    x_h   [N_BLK, P, J, D] f32 : logits shard, token t = i*512 + p*4 + j
    aux_h [P, N_BLK, J, 2] f32 : (ent_mask, nz_mask) per token
    out_h [2, D] f32           : partials (sum_ent x, sum_nz x/||x||)
    repeat: wrap the block loop in a dynamic For_i (timing harness only)
    """
    import concourse.tile as tile
    from concourse import mybir
    from contextlib import nullcontext

    f32 = mybir.dt.float32
    bf16 = mybir.dt.bfloat16
    OP = mybir.AluOpType
    AF = mybir.ActivationFunctionType

    with tile.TileContext(nc) as tc:
        with (
            tc.tile_pool(name="xp", bufs=5) as xp,
            tc.tile_pool(name="xbp", bufs=5) as xbp,
            tc.tile_pool(name="dump", bufs=3) as dumpp,
            tc.tile_pool(name="small", bufs=6) as small,
            tc.tile_pool(name="wp", bufs=4) as wp,
            tc.tile_pool(name="single", bufs=1) as single,
            tc.tile_pool(name="psum", bufs=1, space="PSUM") as psp,
        ):
            aux_sb = single.tile([P, N_BLK, J, 2], f32)
            nc.sync.dma_start(out=aux_sb[:], in_=aux_h[:])

            p512 = psp.tile([2, 512], f32)
            p256 = psp.tile([2, 256], f32)

            loop = tc.For_i(0, repeat, 1) if repeat > 1 else nullcontext()
            with loop:
                for i in range(N_BLK):
                    xb = xbp.tile([P, J, D], bf16)
                    # per-j-slice casting DMAs (gpsimd SWDGE converts
                    # fp32->bf16 in flight): fp32 matmuls stream 4 passes on
                    # the PE (4x cycles); bf16 operands restore 1 col/cycle
                    for jj in range(J):
                        nc.gpsimd.dma_start(out=xb[:, jj, :], in_=x_h[i, :, jj, :])

                    dump = dumpp.tile([P, D], bf16, tag="dump")
                    dump2 = dumpp.tile([P, D], bf16, tag="dump2")
                    sq = small.tile([P, J], f32, tag="sq")
                    for j in range(J):
                        if j < 2:
                            # DVE one-pass square+accumulate (bf16 in, fp32 accum)
                            nc.vector.scalar_tensor_tensor(
                                out=dump[:],
                                in0=xb[:, j, :],
                                scalar=1.0,
                                in1=xb[:, j, :],
                                op0=OP.mult,
                                op1=OP.mult,
                                accum_out=sq[:, j : j + 1],
                            )
                        else:
                            # ACT square+accumulate (parallel engine)
                            nc.scalar.activation(
                                out=dump2[:],
                                in_=xb[:, j, :],
                                func=AF.Square,
                                accum_out=sq[:, j : j + 1],
                            )
                    isq = small.tile([P, J], f32, tag="isq")
                    nc.vector.reciprocal(out=isq[:], in_=sq[:])
                    inv = small.tile([P, J], f32, tag="inv")
                    nc.scalar.activation(out=inv[:], in_=isq[:], func=AF.Sqrt)
                    # per-block weight tile: (ent, nz/||x||) interleaved, bf16
                    w_t = wp.tile([P, J, 2], bf16)
                    nc.scalar.copy(out=w_t[:, :, 0], in_=aux_sb[:, i, :, 0])
                    nc.vector.tensor_tensor(
                        out=w_t[:, :, 1],
                        in0=aux_sb[:, i, :, 1],
                        in1=inv[:],
                        op=OP.mult,
                    )
                    for j in range(J):
                        w = w_t[:, j, :]            # [128, 2]
                        first = i == 0 and j == 0
                        last = i == N_BLK - 1 and j == J - 1
                        nc.tensor.matmul(
                            p512[:], w, xb[:, j, 0:512], start=first, stop=last
                        )
                        nc.tensor.matmul(
                            p256[:], w, xb[:, j, 512:768], start=first, stop=last
                        )

            out_sb = single.tile([2, D], f32)
            nc.vector.tensor_copy(out=out_sb[:, 0:512], in_=p512[:])
            nc.vector.tensor_copy(out=out_sb[:, 512:768], in_=p256[:])
            nc.sync.dma_start(out=out_h[:], in_=out_sb[:])


def _build():
    """Manual module build, used for CoreSim validation only."""
    import concourse.bacc as bacc
    from concourse import mybir

    f32 = mybir.dt.float32
    nc = bacc.Bacc("TRN2", target_bir_lowering=False, debug=False)
    x_dram = nc.dram_tensor("x", [N_BLK, P, J, D], f32, kind="ExternalInput")
    aux_dram = nc.dram_tensor("aux", [P, N_BLK, J, 2], f32, kind="ExternalInput")
    out_dram = nc.dram_tensor("out", [2, D], f32, kind="ExternalOutput")
    _tile_program(nc, x_dram, aux_dram, out_dram)
    nc.finalize()
    return nc


def _get_nc():
    if "nc" not in _CACHE:
        _CACHE["nc"] = _build()
    return _CACHE["nc"]


def _get_sharded_fn():
    """bass_jit kernel shard_mapped over the 8 cores (the proven exec path)."""
    if "fn" in _CACHE:
        return _CACHE["fn"]
    import jax
    from jax.sharding import Mesh, PartitionSpec
    from concourse.bass2jax import bass_jit, bass_shard_map
    from concourse import mybir

    f32 = mybir.dt.float32

    @bass_jit
    def body(nc, x, aux):
        out = nc.dram_tensor("out", [2, D], f32, kind="ExternalOutput")
        _tile_program(nc, x, aux, out)
        return out

    devices = jax.devices()[:N_CORES]
    mesh = Mesh(np.asarray(devices), ("core",))
    fn = bass_shard_map(
        body,
        mesh=mesh,
        in_specs=(PartitionSpec("core"), PartitionSpec("core")),
        out_specs=PartitionSpec("core"),
    )
    _CACHE["fn"] = fn
    return fn


def _make_in_maps(logits, labels, entity_id):
    logits = np.asarray(logits).astype(np.float32, copy=False).reshape(B, S, D)
    labels = np.asarray(labels).reshape(B, S).astype(np.int64, copy=False)
    eid = int(np.asarray(entity_id))

    pos_ok = np.arange(S)[None, :] != 0
    ent = ((labels == eid) & pos_ok).astype(np.float32).reshape(-1)
    nz = (labels != 0).astype(np.float32).reshape(-1)

    in_maps = []
    for c in range(N_CORES):
        shard = logits[c * B_PER_CORE : (c + 1) * B_PER_CORE]
        x = np.ascontiguousarray(shard.reshape(N_BLK, P, J, D))
        sl = slice(c * TOK_PER_CORE, (c + 1) * TOK_PER_CORE)
        ent_c = ent[sl].reshape(N_BLK, P, J)
        nz_c = nz[sl].reshape(N_BLK, P, J)
        aux = np.ascontiguousarray(
            np.stack([ent_c, nz_c], axis=-1).transpose(1, 0, 2, 3)
        )  # [P, N_BLK, J, 2]
        in_maps.append({"x": x, "aux": aux})

    c1 = max(float(ent.sum()), 1.0)
    c2 = max(float(nz.sum()), 1.0)
    return in_maps, c1, c2


def _combine(partials, c1, c2):
    """partials: list of [2, D] float arrays (one per core)."""
    acc = np.zeros((2, D), dtype=np.float64)
    for p in partials:
        acc += np.asarray(p, dtype=np.float64)
    v1, v2 = acc[0], acc[1]
    proto = v1 / c1
    pn = float(np.sqrt((proto * proto).sum()))
    if pn < 1e-30:
        return np.float32(0.0)
    loss = float(v2 @ proto) / (pn * c2)
    return np.float32(loss)


def _run_hw(in_maps):
    """Run the 8-core shard_map; returns list of [2, D] partials."""
    fn = _get_sharded_fn()
    x_g = np.concatenate([m["x"] for m in in_maps], axis=0)
    aux_g = np.concatenate([m["aux"] for m in in_maps], axis=0)
    out = np.asarray(fn(x_g, aux_g))  # [2 * N_CORES, D]
    return [out[2 * c : 2 * c + 2] for c in range(N_CORES)]


def kernel(logits, labels, entity_id):
    in_maps, c1, c2 = _make_in_maps(logits, labels, entity_id)
    partials = _run_hw(in_maps)
    return _combine(partials, c1, c2)



# revision 2
# speedup vs baseline: 1.2177x; 1.2177x over previous
"""Trainium2 Bass kernel for BERTForContrastiveLearningForTokenMetric loss.

Math: the reference loss factors into masked per-token sums:
    proto = (sum_{ent} x_t) / n_ent
    loss  = (sum_{nz} x_t/||x_t||) . proto / (||proto|| * n_tok)
so one pass over logits per core suffices.  Each core processes 8 of the 64
batches (4096 tokens), producing a [2, 768] partial:
    row 0 = sum_{ent tokens} x_t
    row 1 = sum_{nz tokens}  x_t / ||x_t||
The host sums partials across the 8 cores and does the tiny final combine.

Device pipeline (per core), token t = i*512 + p*4 + j:
    8 block DMAs [128, 4*768] issued up-front (all 8 block tiles live in
    SBUF at once - 6.3 MiB of 24), streaming HBM at line rate
    per 512-token block:
        DVE scalar_tensor_tensor (x*x, accum) -> sq[:, j]   j = 0, 1
        ACT Square (accum)                    -> sq[:, j]   j = 2, 3
        DVE reciprocal + ACT sqrt             -> inv = 1/||x||
        DVE tensor_tensor in-place: aux nz slot *= inv      (matmul weights)
        PE  matmul lhsT=aux[:, i, j, :] ([128, 2]) rhs=x -> PSUM [2, 768]
HOST_CAST selects where fp32->bf16 happens: on host (numpy, HWDGE loads)
or in-flight (gpsimd SWDGE cast DMA).  Numerics are identical.
"""

import numpy as np
import ml_dtypes

B, S, D = 64, 512, 768
N_CORES = 8
B_PER_CORE = B // N_CORES            # 8
TOK_PER_CORE = B_PER_CORE * S        # 4096
P = 128                              # SBUF partitions
J = 4                                # tokens per partition per block
BLK_TOK = P * J                      # 512 tokens per block
N_BLK = TOK_PER_CORE // BLK_TOK      # 8

HOST_CAST = True                     # bf16 conversion on host vs in-DMA

_CACHE = {}


def _tile_program(nc, x_h, aux_h, out_h):
    """Emit the per-core Tile program.

    x_h   [N_BLK, P, J, D] f32|bf16 : logits shard, token t = i*512 + p*4 + j
    aux_h [P, N_BLK, J, 2] bf16     : (ent_mask, nz_mask) per token
    out_h [2, D] f32                : partials (sum_ent x, sum_nz x/||x||)
    """
    import concourse.tile as tile
    from concourse import mybir

    f32 = mybir.dt.float32
    bf16 = mybir.dt.bfloat16
    OP = mybir.AluOpType
    AF = mybir.ActivationFunctionType
    cast_on_device = x_h.dtype == f32

    with tile.TileContext(nc) as tc:
        with (
            tc.tile_pool(name="xp", bufs=N_BLK) as xp,
            tc.tile_pool(name="dump", bufs=2) as dumpp,
            tc.tile_pool(name="small", bufs=2) as small,
            tc.tile_pool(name="single", bufs=1) as single,
            tc.tile_pool(name="psum", bufs=1, space="PSUM") as psp,
        ):
            aux_sb = single.tile([P, N_BLK, J, 2], bf16)
            nc.sync.dma_start(out=aux_sb[:], in_=aux_h[:])

            # all 8 block loads queued immediately; SDMA streams back-to-back
            xbs = []
            for i in range(N_BLK):
                xb = xp.tile([P, J, D], bf16)
                xbs.append(xb)
                if cast_on_device:
                    nc.gpsimd.dma_start(out=xb[:], in_=x_h[i])
                else:
                    nc.sync.dma_start(out=xb[:], in_=x_h[i])

            # touch both ACT tables while the first DMA is in flight
            warm = single.tile([P, 2], f32)
            nc.vector.memset(warm[:, 0:1], 1.0)
            nc.scalar.activation(out=warm[:, 1:2], in_=warm[:, 0:1], func=AF.Square)
            nc.scalar.activation(out=warm[:, 0:1], in_=warm[:, 1:2], func=AF.Sqrt)

            p512 = psp.tile([2, 512], f32)
            p256 = psp.tile([2, 256], f32)

            for i in range(N_BLK):
                xb = xbs[i]
                dump = dumpp.tile([P, D], bf16, tag="dump")
                dump2 = dumpp.tile([P, D], bf16, tag="dump2")
                sq = small.tile([P, J], f32, tag="sq")
                for j in range(J):
                    if j < 2:
                        # DVE square+accumulate (bf16 in, fp32 accum)
                        nc.vector.scalar_tensor_tensor(
                            out=dump[:],
                            in0=xb[:, j, :],
                            scalar=1.0,
                            in1=xb[:, j, :],
                            op0=OP.mult,
                            op1=OP.mult,
                            accum_out=sq[:, j : j + 1],
                        )
                    else:
                        # ACT square+accumulate (parallel engine)
                        nc.scalar.activation(
                            out=dump2[:],
                            in_=xb[:, j, :],
                            func=AF.Square,
                            accum_out=sq[:, j : j + 1],
                        )
                isq = small.tile([P, J], f32, tag="isq")
                nc.vector.reciprocal(out=isq[:], in_=sq[:])
                inv = small.tile([P, J], f32, tag="inv")
                nc.scalar.activation(out=inv[:], in_=isq[:], func=AF.Sqrt)
                # nz slot *= 1/||x|| in place: aux[:, i, j, :] is the [128, 2]
                # (ent, nz/||x||) weight pair fed straight to the PE
                nc.vector.tensor_tensor(
                    out=aux_sb[:, i, :, 1],
                    in0=aux_sb[:, i, :, 1],
                    in1=inv[:],
                    op=OP.mult,
                )
                for j in range(J):
                    w = aux_sb[:, i, j, :]      # [128, 2]
                    first = i == 0 and j == 0
                    last = i == N_BLK - 1 and j == J - 1
                    nc.tensor.matmul(
                        p512[:], w, xb[:, j, 0:512], start=first, stop=last
                    )
                    nc.tensor.matmul(
                        p256[:], w, xb[:, j, 512:768], start=first, stop=last
                    )

            out_sb = single.tile([2, D], f32)
            nc.vector.tensor_copy(out=out_sb[:, 0:512], in_=p512[:])
            nc.scalar.copy(out=out_sb[:, 512:768], in_=p256[:])
            nc.sync.dma_start(out=out_h[:], in_=out_sb[:])


def _x_dtype(mybir):
    return mybir.dt.bfloat16 if HOST_CAST else mybir.dt.float32


def _build():
    """Manual module build, used for CoreSim validation and timing."""
    import concourse.bacc as bacc
    from concourse import mybir

    f32 = mybir.dt.float32
    bf16 = mybir.dt.bfloat16
    nc = bacc.Bacc("TRN2", target_bir_lowering=False, debug=False)
    x_dram = nc.dram_tensor("x", [N_BLK, P, J, D], _x_dtype(mybir), kind="ExternalInput")
    aux_dram = nc.dram_tensor("aux", [P, N_BLK, J, 2], bf16, kind="ExternalInput")
    out_dram = nc.dram_tensor("out", [2, D], f32, kind="ExternalOutput")
    _tile_program(nc, x_dram, aux_dram, out_dram)
    nc.finalize()
    return nc


def _get_nc():
    if "nc" not in _CACHE:
        _CACHE["nc"] = _build()
    return _CACHE["nc"]


def _get_sharded_fn():
    """bass_jit kernel shard_mapped over the 8 cores (the proven exec path)."""
    if "fn" in _CACHE:
        return _CACHE["fn"]
    import jax
    from jax.sharding import Mesh, PartitionSpec
    from concourse.bass2jax import bass_jit, bass_shard_map
    from concourse import mybir

    f32 = mybir.dt.float32

    @bass_jit
    def body(nc, x, aux):
        out = nc.dram_tensor("out", [2, D], f32, kind="ExternalOutput")
        _tile_program(nc, x, aux, out)
        return out

    devices = jax.devices()[:N_CORES]
    mesh = Mesh(np.asarray(devices), ("core",))
    fn = bass_shard_map(
        body,
        mesh=mesh,
        in_specs=(PartitionSpec("core"), PartitionSpec("core")),
        out_specs=PartitionSpec("core"),
    )
    _CACHE["fn"] = fn
    return fn


def _make_in_maps(logits, labels, entity_id):
    logits = np.asarray(logits).astype(np.float32, copy=False).reshape(B, S, D)
    labels = np.asarray(labels).reshape(B, S).astype(np.int64, copy=False)
    eid = int(np.asarray(entity_id))

    pos_ok = np.arange(S)[None, :] != 0
    ent = ((labels == eid) & pos_ok).astype(np.float32).reshape(-1)
    nz = (labels != 0).astype(np.float32).reshape(-1)

    if HOST_CAST:
        x_all = logits.reshape(N_CORES, N_BLK, P, J, D).astype(ml_dtypes.bfloat16)
    else:
        x_all = logits.reshape(N_CORES, N_BLK, P, J, D)

    in_maps = []
    for c in range(N_CORES):
        x = np.ascontiguousarray(x_all[c])
        sl = slice(c * TOK_PER_CORE, (c + 1) * TOK_PER_CORE)
        ent_c = ent[sl].reshape(N_BLK, P, J)
        nz_c = nz[sl].reshape(N_BLK, P, J)
        aux = np.ascontiguousarray(
            np.stack([ent_c, nz_c], axis=-1).transpose(1, 0, 2, 3)
        ).astype(ml_dtypes.bfloat16)  # [P, N_BLK, J, 2]
        in_maps.append({"x": x, "aux": aux})

    c1 = max(float(ent.sum()), 1.0)
    c2 = max(float(nz.sum()), 1.0)
    return in_maps, c1, c2


def _combine(partials, c1, c2):
    """partials: list of [2, D] float arrays (one per core)."""
    acc = np.zeros((2, D), dtype=np.float64)
    for p in partials:
        acc += np.asarray(p, dtype=np.float64)
    v1, v2 = acc[0], acc[1]
    proto = v1 / c1
    pn = float(np.sqrt((proto * proto).sum()))
    if pn < 1e-30:
        return np.float32(0.0)
    loss = float(v2 @ proto) / (pn * c2)
    return np.float32(loss)


def _run_hw(in_maps):
    """Run the 8-core shard_map; returns list of [2, D] partials."""
    fn = _get_sharded_fn()
    x_g = np.concatenate([m["x"] for m in in_maps], axis=0)
    aux_g = np.concatenate([m["aux"] for m in in_maps], axis=0)
    out = np.asarray(fn(x_g, aux_g))  # [2 * N_CORES, D]
    return [out[2 * c : 2 * c + 2] for c in range(N_CORES)]


def kernel(logits, labels, entity_id):
    in_maps, c1, c2 = _make_in_maps(logits, labels, entity_id)
    partials = _run_hw(in_maps)
    return _combine(partials, c1, c2)


# revision 7
# speedup vs baseline: 1.2238x; 1.0050x over previous
"""Trainium2 Bass kernel for BERTForContrastiveLearningForTokenMetric loss.

Math: the reference loss factors into masked per-token sums:
    proto = (sum_{ent} x_t) / n_ent
    loss  = (sum_{nz} x_t/||x_t||) . proto / (||proto|| * n_tok)
so one pass over logits per core suffices.  Each core processes 8 of the 64
batches (4096 tokens), producing a [2, 768] partial:
    row 0 = sum_{ent tokens} x_t
    row 1 = sum_{nz tokens}  x_t / ||x_t||
The host sums partials across the 8 cores and does the tiny final combine.

Device pipeline (per core), token t = i*512 + p*4 + j:
    8 block DMAs [128, 4*768] issued up-front (all 8 block tiles live in
    SBUF at once - 6.3 MiB of 24), streaming HBM at line rate
    per 512-token block:
        DVE scalar_tensor_tensor (x*x, accum) -> sq[:, j]   j = 0, 1
        ACT Square (accum)                    -> sq[:, j]   j = 2, 3
        DVE reciprocal + ACT sqrt             -> inv = 1/||x||
        DVE tensor_tensor in-place: aux nz slot *= inv      (matmul weights)
        PE  matmul lhsT=aux[:, i, j, :] ([128, 2]) rhs=x -> PSUM [2, 768]
HOST_CAST selects where fp32->bf16 happens: on host (numpy, HWDGE loads)
or in-flight (gpsimd SWDGE cast DMA).  Numerics are identical.
"""

import numpy as np
import ml_dtypes

B, S, D = 64, 512, 768
N_CORES = 8
B_PER_CORE = B // N_CORES            # 8
TOK_PER_CORE = B_PER_CORE * S        # 4096
P = 128                              # SBUF partitions
J = 4                                # tokens per partition per block
BLK_TOK = P * J                      # 512 tokens per block
N_BLK = TOK_PER_CORE // BLK_TOK      # 8

HOST_CAST = True                     # bf16 conversion on host vs in-DMA

_CACHE = {}


def _tile_program(nc, x_h, aux_h, out_h):
    """Emit the per-core Tile program.

    x_h   [N_BLK, P, J, D] f32|bf16 : logits shard, token t = i*512 + p*4 + j
    aux_h [P, N_BLK, J, 2] bf16     : (ent_mask, nz_mask) per token
    out_h [2, D] f32                : partials (sum_ent x, sum_nz x/||x||)
    """
    import concourse.tile as tile
    from concourse import mybir

    f32 = mybir.dt.float32
    bf16 = mybir.dt.bfloat16
    OP = mybir.AluOpType
    AF = mybir.ActivationFunctionType
    cast_on_device = x_h.dtype == f32

    # square-slice owner per (block, j): DVE or ACT, two slices each
    # (Pool/gpsimd rejects the scalar_tensor_tensor opcode on TRN2)
    def sq_engine(i, j):
        return ("V", "A", "A", "V")[j]

    with tile.TileContext(nc) as tc:
        with (
            tc.tile_pool(name="xp", bufs=N_BLK) as xp,
            tc.tile_pool(name="dump", bufs=2) as dumpp,
            tc.tile_pool(name="small", bufs=2) as small,
            tc.tile_pool(name="single", bufs=1) as single,
            tc.tile_pool(name="psum", bufs=1, space="PSUM") as psp,
        ):
            aux_sb = single.tile([P, N_BLK, J, 2], bf16)
            nc.sync.dma_start(out=aux_sb[:], in_=aux_h[:])

            # Block loads, queued up-front; all 8 block tiles stay live.
            # Block 0 lands as 4 per-j slices so compute starts early.
            # Issue is split between sync (HWDGE) and gpsimd (SWDGE) rings
            # so descriptor generation never paces the stream (device-cast
            # requires SWDGE, so that path keeps everything on gpsimd).
            xbs = []
            for i in range(N_BLK):
                xb = xp.tile([P, J, D], bf16)
                xbs.append(xb)
                eng = nc.gpsimd if (cast_on_device or i >= 5) else nc.sync
                if i == 0:
                    for j in range(J):
                        eng.dma_start(out=xb[:, j, :], in_=x_h[i, :, j, :])
                else:
                    eng.dma_start(out=xb[:], in_=x_h[i])

            # touch both ACT tables while the first DMA is in flight
            warm = single.tile([P, 2], f32)
            nc.vector.memset(warm[:, 0:1], 1.0)
            nc.scalar.activation(out=warm[:, 1:2], in_=warm[:, 0:1], func=AF.Square)
            nc.scalar.activation(out=warm[:, 0:1], in_=warm[:, 1:2], func=AF.Sqrt)

            p512 = psp.tile([2, 512], f32)
            p256 = psp.tile([2, 256], f32)

            def square(i, j, xb, sq, dumps):
                e = sq_engine(i, j)
                if e == "A":
                    nc.scalar.activation(
                        out=dumps["A"][:],
                        in_=xb[:, j, :],
                        func=AF.Square,
                        accum_out=sq[:, j : j + 1],
                    )
                else:
                    eng = nc.vector
                    eng.scalar_tensor_tensor(
                        out=dumps[e][:],
                        in0=xb[:, j, :],
                        scalar=1.0,
                        in1=xb[:, j, :],
                        op0=OP.mult,
                        op1=OP.mult,
                        accum_out=sq[:, j : j + 1],
                    )

            def weights(i, sq, isq, inv, j0, j1):
                """recip+sqrt+mask-multiply for j slice [j0, j1)."""
                s = slice(j0, j1)
                nc.vector.reciprocal(out=isq[:, s], in_=sq[:, s])
                nc.scalar.activation(out=inv[:, s], in_=isq[:, s], func=AF.Sqrt)
                nc.vector.tensor_tensor(
                    out=aux_sb[:, i, s, 1],
                    in0=aux_sb[:, i, s, 1],
                    in1=inv[:, s],
                    op=OP.mult,
                )

            def matmuls(i, j, xb):
                w = aux_sb[:, i, j, :]          # [128, 2]
                first = i == 0 and j == 0
                last = i == N_BLK - 1 and j == J - 1
                nc.tensor.matmul(p512[:], w, xb[:, j, 0:512], start=first, stop=last)
                nc.tensor.matmul(p256[:], w, xb[:, j, 512:768], start=first, stop=last)

            for i in range(N_BLK):
                xb = xbs[i]
                dump_v = dumpp.tile([P, D], bf16, tag="dumpV")
                dump_a = dumpp.tile([P, D], bf16, tag="dumpA")
                dump_g = dumpp.tile([P, D], bf16, tag="dumpG")
                dumps = {"V": dump_v, "A": dump_a, "G": dump_g}
                sq = small.tile([P, J], f32, tag="sq")
                isq = small.tile([P, J], f32, tag="isq")
                inv = small.tile([P, J], f32, tag="inv")
                if i == 0 or i == N_BLK - 1:
                    # half-block chains: short pipeline at start and finish
                    for h in range(2):
                        for j in (2 * h, 2 * h + 1):
                            square(i, j, xb, sq, dumps)
                        weights(i, sq, isq, inv, 2 * h, 2 * h + 2)
                        for j in (2 * h, 2 * h + 1):
                            matmuls(i, j, xb)
                else:
                    for j in range(J):
                        square(i, j, xb, sq, dumps)
                    weights(i, sq, isq, inv, 0, J)
                    for j in range(J):
                        matmuls(i, j, xb)

            out_sb = single.tile([2, D], f32)
            nc.vector.tensor_copy(out=out_sb[:, 0:512], in_=p512[:])
            nc.scalar.copy(out=out_sb[:, 512:768], in_=p256[:])
            nc.sync.dma_start(out=out_h[:], in_=out_sb[:])


def _x_dtype(mybir):
    return mybir.dt.bfloat16 if HOST_CAST else mybir.dt.float32


def _build():
    """Manual module build, used for CoreSim validation and timing."""
    import concourse.bacc as bacc
    from concourse import mybir

    f32 = mybir.dt.float32
    bf16 = mybir.dt.bfloat16
    nc = bacc.Bacc("TRN2", target_bir_lowering=False, debug=False)
    x_dram = nc.dram_tensor("x", [N_BLK, P, J, D], _x_dtype(mybir), kind="ExternalInput")
    aux_dram = nc.dram_tensor("aux", [P, N_BLK, J, 2], bf16, kind="ExternalInput")
    out_dram = nc.dram_tensor("out", [2, D], f32, kind="ExternalOutput")
    _tile_program(nc, x_dram, aux_dram, out_dram)
    nc.finalize()
    return nc


def _get_nc():
    if "nc" not in _CACHE:
        _CACHE["nc"] = _build()
    return _CACHE["nc"]


def _get_sharded_fn():
    """bass_jit kernel shard_mapped over the 8 cores (the proven exec path)."""
    if "fn" in _CACHE:
        return _CACHE["fn"]
    import jax
    from jax.sharding import Mesh, PartitionSpec
    from concourse.bass2jax import bass_jit, bass_shard_map
    from concourse import mybir

    f32 = mybir.dt.float32

    @bass_jit
    def body(nc, x, aux):
        out = nc.dram_tensor("out", [2, D], f32, kind="ExternalOutput")
        _tile_program(nc, x, aux, out)
        return out

    devices = jax.devices()[:N_CORES]
    mesh = Mesh(np.asarray(devices), ("core",))
    fn = bass_shard_map(
        body,
        mesh=mesh,
        in_specs=(PartitionSpec("core"), PartitionSpec("core")),
        out_specs=PartitionSpec("core"),
    )
    _CACHE["fn"] = fn
    return fn


def _make_in_maps(logits, labels, entity_id):
    logits = np.asarray(logits).astype(np.float32, copy=False).reshape(B, S, D)
    labels = np.asarray(labels).reshape(B, S).astype(np.int64, copy=False)
    eid = int(np.asarray(entity_id))

    pos_ok = np.arange(S)[None, :] != 0
    ent = ((labels == eid) & pos_ok).astype(np.float32).reshape(-1)
    nz = (labels != 0).astype(np.float32).reshape(-1)

    if HOST_CAST:
        x_all = logits.reshape(N_CORES, N_BLK, P, J, D).astype(ml_dtypes.bfloat16)
    else:
        x_all = logits.reshape(N_CORES, N_BLK, P, J, D)

    in_maps = []
    for c in range(N_CORES):
        x = np.ascontiguousarray(x_all[c])
        sl = slice(c * TOK_PER_CORE, (c + 1) * TOK_PER_CORE)
        ent_c = ent[sl].reshape(N_BLK, P, J)
        nz_c = nz[sl].reshape(N_BLK, P, J)
        aux = np.ascontiguousarray(
            np.stack([ent_c, nz_c], axis=-1).transpose(1, 0, 2, 3)
        ).astype(ml_dtypes.bfloat16)  # [P, N_BLK, J, 2]
        in_maps.append({"x": x, "aux": aux})

    c1 = max(float(ent.sum()), 1.0)
    c2 = max(float(nz.sum()), 1.0)
    return in_maps, c1, c2


def _combine(partials, c1, c2):
    """partials: list of [2, D] float arrays (one per core)."""
    acc = np.zeros((2, D), dtype=np.float64)
    for p in partials:
        acc += np.asarray(p, dtype=np.float64)
    v1, v2 = acc[0], acc[1]
    proto = v1 / c1
    pn = float(np.sqrt((proto * proto).sum()))
    if pn < 1e-30:
        return np.float32(0.0)
    loss = float(v2 @ proto) / (pn * c2)
    return np.float32(loss)


def _run_hw(in_maps):
    """Run the 8-core shard_map; returns list of [2, D] partials."""
    fn = _get_sharded_fn()
    x_g = np.concatenate([m["x"] for m in in_maps], axis=0)
    aux_g = np.concatenate([m["aux"] for m in in_maps], axis=0)
    out = np.asarray(fn(x_g, aux_g))  # [2 * N_CORES, D]
    return [out[2 * c : 2 * c + 2] for c in range(N_CORES)]


def kernel(logits, labels, entity_id):
    in_maps, c1, c2 = _make_in_maps(logits, labels, entity_id)
    partials = _run_hw(in_maps)
    return _combine(partials, c1, c2)
